# revision 3
# baseline (speedup 1.0000x reference)
"""Dot-product attention on 8 Trainium2 NeuronCores — v6 (3-engine exp).

Full inputs [B=4, H=16, S=1024, D=64] fp32; 64 heads sharded 8 per core,
processed sequentially.  The kernel is pointwise-bound: every score
element must cross PSUM(fp32)->SBUF once through ScalarE (1/cyc @1.2GHz)
or VectorE (1/cyc @0.96GHz for fp32 src).  v6 balances that pass across
ScalarE (true exp, ~42 of 64 ki-tiles) and VectorE (dual-phase
Schraudolph, ~22 tiles) with GpSimdE absorbing the int16 middle step,
and removes the v5 pipeline bubbles:
  - psO (output accumulator) double-buffered (2x2 PSUM banks) so head
    h+1's MM2 no longer waits for head h's drain       (was ~1us/head)
  - score slots reduced 3->2 (2x2 banks) to fund psO   (8 banks total)
  - head 0's kq DMA split across 4 queues              (was ~4us serial)
  - last head's exp tiles split evenly + split drain   (shorter tail)

Per head h, per ki (8 k-tiles of 128):
  stage[k,q] = K^T d-major @ Q^T d-major   (fp16; Q/K duplicated in both
                                            PE row halves so the two
                                            q-half matmuls run in
                                            parallel row groups)
  E = ~exp(stage/8) fp16                   ScalarE true exp  OR
                                           VectorE A=rint(a*s+b) i16 ->
                                           GpSimdE B=A-512 ->
                                           VectorE E=f16(A)+f16(B)
  O^T+sums += [V | 1]^T @ E                (fp16, lags exp by 2 ki)
  drain: VectorE copy psum -> fp16 SBUF -> DMA out (unnormalized + sums)
Host: out[q,d] = (O^T[d,q] / sums[q])^T while gathering shards.

Toolchain notes (walrus 2026-05-04 + bass_rust skew):
 - walrus accepts at most ONE sync-wait per instruction; a JSON pass over
   the BIR hoists extra waits onto NoOps (same engine, in-order).
 - lower_dve crashes with ldw-opt enabled; keep it off.
"""

import json
from contextlib import ExitStack

import numpy as np

import concourse.bass as bass
import concourse.bass2jax as bass2jax
import concourse.mybir as mybir
import concourse.tile as tile
from concourse import bass_utils
from concourse.vector_clock import ScopedClock

F32 = mybir.dt.float32
F16 = mybir.dt.float16
I16 = mybir.dt.int16
Alu = mybir.AluOpType

N_CORES = 8
HEADS_PER_CORE = 8
S = 1024
D = 64
KT = S // 128  # 8 k-tiles per head

SCALE = 0.125  # 1/sqrt(64)

# dual-phase Schraudolph constants: A = rint(a1*s + b1) int16;
# E = fp16_bits(A) + fp16_bits(A-512) approximates exp(s*SCALE)
# with max rel err ~1.1% and unit gain.
EXP_A1 = 184.6649627685547
EXP_B1 = 14517.731933593746

# which ki-stages per head use the VectorE+GpSimd fast-exp path
DVE_KIS = {
    0: (0, 1, 4, 6),
    1: (2, 5),
    2: (2, 5, 7),
    3: (2, 5),
    4: (2, 5, 7),
    5: (2, 5),
    6: (2, 5, 7),
    7: (1, 3, 5),
}
LAG = 2  # MM2 trails exp by this many ki

_DRAIN_MAX_WAITS = 1


def _split_drain_and_barrier(self, tick_clock, wait_clock):
    nc = self.nc
    drain_inst = nc.sync.drain()
    wait_clock.add_sem_waits(
        drain_inst.ins, ScopedClock({None: tick_clock.global_clock})
    )
    si = drain_inst.ins.sync_info
    if si is not None and si.on_wait and len(si.on_wait) > _DRAIN_MAX_WAITS:
        waits = list(si.on_wait)
        updates = list(si.on_update or [])
        drain_inst.ins.sync_info = mybir.SyncInfo(
            on_wait=waits[:_DRAIN_MAX_WAITS], on_update=[]
        )
        rest = waits[_DRAIN_MAX_WAITS:]
        for i in range(0, len(rest), _DRAIN_MAX_WAITS):
            extra = nc.sync.drain()
            extra.ins.sync_info = mybir.SyncInfo(
                on_wait=rest[i : i + _DRAIN_MAX_WAITS],
                on_update=updates if i + _DRAIN_MAX_WAITS >= len(rest) else [],
            )
    nc.all_engine_barrier()
    assert self.sems is not None
    popped = nc._tile_sem_poison_stack.pop()
    assert popped is self._sem_poison
    nc.clear_and_free_semaphores(list(self.sems.allocated().values()))
    nc.all_engine_barrier()


def _split_waits_in_bir(bir_json: bytes) -> bytes:
    """Hoist extra sync-waits onto NoOps inserted immediately before the
    owning instruction (same engine, in-order => semantics unchanged)."""
    j = json.loads(bir_json)
    n = 0
    for f in j["functions"]:
        for b in f["blocks"]:
            out = []
            for inst in b["instructions"]:
                si = inst.get("sync_info")
                waits = (si or {}).get("on_wait") or []
                if len(waits) > 1:
                    for w in waits[:-1]:
                        out.append(
                            {
                                "debug": inst.get("debug", 0),
                                "engine": inst["engine"],
                                "ins": [],
                                "outs": [],
                                "name": f"{inst['name']}-wsplit{n}",
                                "opcode": "NoOp",
                                "sync_info": {"on_update": [], "on_wait": [w]},
                            }
                        )
                        n += 1
                    si["on_wait"] = [waits[-1]]
                out.append(inst)
            b["instructions"] = out
    return json.dumps(j).encode()


_orig_compile_bir_kernel = bass_utils.compile_bir_kernel


def _compile_bir_kernel_splitting(bir_json, tmpdir, neff_name="file.neff"):
    return _orig_compile_bir_kernel(_split_waits_in_bir(bir_json), tmpdir, neff_name)


ENABLE_LDW_OPT = False
_orig_run_command = bass_utils.run_command


def _run_command_ldw(argv, **kwargs):
    if ENABLE_LDW_OPT:
        argv = [
            a.replace("--enable-ldw-opt=false", "--enable-ldw-opt=true") for a in argv
        ]
    return _orig_run_command(argv, **kwargs)


def _install_patches():
    if not getattr(tile.TileContext, "_drain_split_installed", False):
        tile.TileContext._drain_and_barrier = _split_drain_and_barrier
        tile.TileContext._drain_split_installed = True
    if bass_utils.compile_bir_kernel is not _compile_bir_kernel_splitting:
        bass_utils.compile_bir_kernel = _compile_bir_kernel_splitting
        bass2jax.compile_bir_kernel = _compile_bir_kernel_splitting
        bass_utils.run_command = _run_command_ldw


def build_nc() -> bass.Bass:
    _install_patches()
    nc = bass.Bass(
        trn_type="TRN2", target_bir_lowering=False, debug=False, num_devices=N_CORES
    )
    # kq[h, 0:64, 0:1024] = Q^T head h ; [0:64, 1024:] = K^T head h
    # kq[h, 64:128, ...]   = the SAME data duplicated (row-group packing)
    kq = nc.dram_tensor(
        "kq", [HEADS_PER_CORE, 128, 2 * S], F16, kind="ExternalInput"
    ).ap()
    # vext[h, p, t, j]: V[h, 128*t + p, j] for j < 64, 1.0 at j == 64 (fp16)
    vext = nc.dram_tensor(
        "vext", [HEADS_PER_CORE, 128, KT, 65], F16, kind="ExternalInput"
    ).ap()
    # outu[h, 0:64, q] = unnormalized out^T ; outu[h, 64, q] = softmax sums
    outu = nc.dram_tensor(
        "outu", [HEADS_PER_CORE, 65, S], F16, kind="ExternalOutput"
    ).ap()

    with tile.TileContext(nc) as tc, ExitStack() as ctx:
        sb = ctx.enter_context(tc.tile_pool(name="sb", bufs=2))
        psS = ctx.enter_context(tc.tile_pool(name="psS", bufs=2, space="PSUM"))
        psO = ctx.enter_context(tc.tile_pool(name="psO", bufs=2, space="PSUM"))

        # pay the one-time ACT table load while the first DMAs stream
        singles = ctx.enter_context(tc.tile_pool(name="singles", bufs=1))
        warm_in = singles.tile([128, 16], F32, tag="warm_in")
        nc.vector.memset(warm_in, 0.0)
        warm_out = singles.tile([128, 16], F16, tag="warm_out")
        nc.scalar.activation(out=warm_out, in_=warm_in,
                             func=mybir.ActivationFunctionType.Exp, scale=1.0)

        kq_tiles = {}
        v_tiles = {}

        def prefetch_kq0():
            kq_s = sb.tile([128, 2 * S], F16, tag="kq", name="kq_0")
            nc.sync.dma_start(kq_s[:, 0:768], kq[0][:, 0:768])
            nc.scalar.dma_start(kq_s[:, 768:1536], kq[0][:, 768:1536])
            nc.gpsimd.dma_start(kq_s[:, 1536:2048], kq[0][:, 1536:2048])
            kq_tiles[0] = kq_s

        def prefetch_kq(h):
            kq_s = sb.tile([128, 2 * S], F16, tag="kq", name=f"kq_{h}")
            nc.sync.dma_start(kq_s[:, : S + 128], kq[h][:, : S + 128])
            nc.gpsimd.dma_start(kq_s[:, S + 128 :], kq[h][:, S + 128 :])
            kq_tiles[h] = kq_s

        def prefetch_v(h):
            v_s = sb.tile([128, KT, 65], F16, tag="v", name=f"v_{h}")
            nc.gpsimd.dma_start(v_s, vext[h])
            v_tiles[h] = v_s

        prefetch_kq0()
        prefetch_v(0)

        for h in range(HEADS_PER_CORE):
            kq_s = kq_tiles.pop(h)
            v_s = v_tiles.pop(h)
            e_s = sb.tile([128, KT, 2 * 512], F16, tag="e", name=f"e_{h}")
            dve_kis = DVE_KIS[h]
            o_ps = psO.tile([65, 2 * 512], F32, tag="o", name=f"o_{h}")

            def mm2(kj):
                for c in range(2):
                    nc.tensor.matmul(
                        o_ps[:, c * 512 : (c + 1) * 512],
                        v_s[:, kj, :],
                        e_s[:, kj, c * 512 : (c + 1) * 512],
                        start=(kj == 0),
                        stop=(kj == KT - 1),
                    )

            for ki in range(KT):
                st = psS.tile([128, 2 * 512], F32, tag="st", name=f"st_{h}_{ki}")
                for c in range(2):
                    b0 = 64 * c
                    nc.tensor.matmul(
                        st[:, c * 512 : (c + 1) * 512],
                        kq_s[b0 : b0 + 64, S + ki * 128 : S + (ki + 1) * 128],
                        kq_s[b0 : b0 + 64, c * 512 : (c + 1) * 512],
                        start=True,
                        stop=True,
                        tile_position=(b0, 0),
                    )
                dst = e_s[:, ki]
                if ki in dve_kis:
                    a_t = sb.tile([128, 1024], F16, tag="exp_a")
                    nc.vector.tensor_scalar(
                        out=a_t.bitcast(I16), in0=st, scalar1=EXP_A1,
                        scalar2=EXP_B1, op0=Alu.mult, op1=Alu.add,
                    )
                    b_t = sb.tile([128, 1024], F16, tag="exp_b")
                    nc.gpsimd.tensor_scalar(
                        out=b_t.bitcast(I16), in0=a_t.bitcast(I16),
                        scalar1=-512, scalar2=None, op0=Alu.add,
                    )
                    nc.vector.tensor_add(dst, a_t, b_t)
                else:
                    nc.scalar.activation(
                        out=dst, in_=st,
                        func=mybir.ActivationFunctionType.Exp, scale=SCALE,
                    )
                if ki == 1:
                    if h + 1 < HEADS_PER_CORE:
                        prefetch_v(h + 1)
                elif ki == 2:
                    if h + 1 < HEADS_PER_CORE:
                        prefetch_kq(h + 1)
                if ki >= LAG:
                    mm2(ki - LAG)
            for kj in range(KT - LAG, KT):
                mm2(kj)

            ou = sb.tile([65, 2 * 512], F16, tag="ou", name=f"ou_{h}")
            if h == HEADS_PER_CORE - 1:
                # split the last drain across both engines to shorten the tail
                nc.scalar.copy(out=ou[:, :512], in_=o_ps[:, :512])
                nc.vector.tensor_copy(ou[:, 512:], o_ps[:, 512:])
            else:
                nc.vector.tensor_copy(ou, o_ps)
            nc.sync.dma_start(outu[h], ou)

    return nc


def _shard_inputs(queries, keys, values):
    """Full [4,16,1024,64] fp32 -> per-core kq / vext (fp16)."""
    q = np.ascontiguousarray(queries, dtype=np.float32).reshape(64, S, D)
    k = np.ascontiguousarray(keys, dtype=np.float32).reshape(64, S, D)
    v = np.ascontiguousarray(values, dtype=np.float32).reshape(64, S, D)

    qT = q.transpose(0, 2, 1)  # [64, D, S]
    kT = k.transpose(0, 2, 1)

    kq = np.empty((64, 128, 2 * S), np.float16)
    kq[:, 0:64, 0:S] = qT
    kq[:, 0:64, S:] = kT
    kq[:, 64:128, 0:S] = qT
    kq[:, 64:128, S:] = kT

    vext = np.empty((64, 128, KT, 65), np.float16)
    vext[..., 64] = 1.0
    vext[..., :64] = v.reshape(64, KT, 128, D).transpose(0, 2, 1, 3)

    in_maps = []
    for c in range(N_CORES):
        in_maps.append(
            {
                "kq": np.ascontiguousarray(kq[c * 8 : (c + 1) * 8]),
                "vext": np.ascontiguousarray(vext[c * 8 : (c + 1) * 8]),
            }
        )
    return in_maps


_CACHE = {}


def _get_nc() -> bass.Bass:
    if "nc" not in _CACHE:
        _CACHE["nc"] = build_nc()
    return _CACHE["nc"]


def run(queries, keys, values, d_k, trace=False, trace_kwargs=None):
    assert int(d_k) == D
    nc = _get_nc()
    in_maps = _shard_inputs(queries, keys, values)
    res = bass_utils.run_bass_kernel_spmd(
        nc,
        in_maps,
        core_ids=list(range(N_CORES)),
        trace=trace,
        **(trace_kwargs or {}),
    )
    outu = np.stack([r["outu"] for r in res.results]).astype(np.float32)
    # [8 cores, 8 heads, 65, S] -> normalize + transpose
    outu = outu.reshape(64, 65, S)
    out = outu[:, 0:64, :] / outu[:, 64:65, :]  # [64, D, S]
    out = np.ascontiguousarray(out.transpose(0, 2, 1)).reshape(4, 16, S, D)
    return out.astype(np.float32), res


def kernel(queries, keys, values, d_k):
    out, _ = run(queries, keys, values, d_k, trace=False)
    return out


# revision 7
# speedup vs baseline: 4.4643x; 4.4643x over previous
"""Dot-product attention on 8 Trainium2 NeuronCores — v6 (3-engine exp).

Full inputs [B=4, H=16, S=1024, D=64] fp32; 64 heads sharded 8 per core,
processed sequentially.  The kernel is pointwise-bound: every score
element must cross PSUM(fp32)->SBUF once through ScalarE (1/cyc @1.2GHz)
or VectorE (1/cyc @0.96GHz for fp32 src).  v6 balances that pass across
ScalarE (true exp, ~42 of 64 ki-tiles) and VectorE (dual-phase
Schraudolph, ~22 tiles) with GpSimdE absorbing the int16 middle step,
and removes the v5 pipeline bubbles:
  - psO (output accumulator) double-buffered (2x2 PSUM banks) so head
    h+1's MM2 no longer waits for head h's drain       (was ~1us/head)
  - score slots reduced 3->2 (2x2 banks) to fund psO   (8 banks total)
  - head 0's kq DMA split across 4 queues              (was ~4us serial)
  - last head's exp tiles split evenly + split drain   (shorter tail)

Per head h, per ki (8 k-tiles of 128):
  stage[k,q] = K^T d-major @ Q^T d-major   (fp16; Q/K duplicated in both
                                            PE row halves so the two
                                            q-half matmuls run in
                                            parallel row groups)
  E = ~exp(stage/8) fp16                   ScalarE true exp  OR
                                           VectorE A=rint(a*s+b) i16 ->
                                           GpSimdE B=A-512 ->
                                           VectorE E=f16(A)+f16(B)
  O^T+sums += [V | 1]^T @ E                (fp16, lags exp by 2 ki)
  drain: VectorE copy psum -> fp16 SBUF -> DMA out (unnormalized + sums)
Host: out[q,d] = (O^T[d,q] / sums[q])^T while gathering shards.

Toolchain notes (walrus 2026-05-04 + bass_rust skew):
 - walrus accepts at most ONE sync-wait per instruction; a JSON pass over
   the BIR hoists extra waits onto NoOps (same engine, in-order).
 - lower_dve crashes with ldw-opt enabled; keep it off.
"""

import json
from contextlib import ExitStack

import numpy as np

import concourse.bass as bass
import concourse.bass2jax as bass2jax
import concourse.mybir as mybir
import concourse.tile as tile
from concourse import bass_utils
from concourse.vector_clock import ScopedClock

F32 = mybir.dt.float32
F16 = mybir.dt.float16
I16 = mybir.dt.int16
Alu = mybir.AluOpType

N_CORES = 8
HEADS_PER_CORE = 8
S = 1024
D = 64
KT = S // 128  # 8 k-tiles per head

SCALE = 0.125  # 1/sqrt(64)

# dual-phase Schraudolph constants: A = rint(a1*s + b1) int16;
# E = fp16_bits(A) + fp16_bits(A-512) approximates exp(s*SCALE)
# with max rel err ~1.1% and unit gain.
EXP_A1 = 184.6649627685547
EXP_B1 = 14517.731933593746

# which ki-stages per head use the VectorE fast-exp path
DVE_KIS = {
    0: (0, 1, 4),
    1: (2, 6),
    2: (2, 6),
    3: (2, 6),
    4: (2, 6),
    5: (2, 6),
    6: (2, 6),
    7: (1, 3, 5),
}
LAG = 2  # MM2 trails exp by this many ki

_DRAIN_MAX_WAITS = 1


def _split_drain_and_barrier(self, tick_clock, wait_clock):
    nc = self.nc
    drain_inst = nc.sync.drain()
    wait_clock.add_sem_waits(
        drain_inst.ins, ScopedClock({None: tick_clock.global_clock})
    )
    si = drain_inst.ins.sync_info
    if si is not None and si.on_wait and len(si.on_wait) > _DRAIN_MAX_WAITS:
        waits = list(si.on_wait)
        updates = list(si.on_update or [])
        drain_inst.ins.sync_info = mybir.SyncInfo(
            on_wait=waits[:_DRAIN_MAX_WAITS], on_update=[]
        )
        rest = waits[_DRAIN_MAX_WAITS:]
        for i in range(0, len(rest), _DRAIN_MAX_WAITS):
            extra = nc.sync.drain()
            extra.ins.sync_info = mybir.SyncInfo(
                on_wait=rest[i : i + _DRAIN_MAX_WAITS],
                on_update=updates if i + _DRAIN_MAX_WAITS >= len(rest) else [],
            )
    nc.all_engine_barrier()
    assert self.sems is not None
    popped = nc._tile_sem_poison_stack.pop()
    assert popped is self._sem_poison
    nc.clear_and_free_semaphores(list(self.sems.allocated().values()))
    nc.all_engine_barrier()


def _split_waits_in_bir(bir_json: bytes) -> bytes:
    """Hoist extra sync-waits onto NoOps inserted immediately before the
    owning instruction (same engine, in-order => semantics unchanged)."""
    j = json.loads(bir_json)
    n = 0
    for f in j["functions"]:
        for b in f["blocks"]:
            out = []
            for inst in b["instructions"]:
                si = inst.get("sync_info")
                waits = (si or {}).get("on_wait") or []
                if len(waits) > 1:
                    for w in waits[:-1]:
                        out.append(
                            {
                                "debug": inst.get("debug", 0),
                                "engine": inst["engine"],
                                "ins": [],
                                "outs": [],
                                "name": f"{inst['name']}-wsplit{n}",
                                "opcode": "NoOp",
                                "sync_info": {"on_update": [], "on_wait": [w]},
                            }
                        )
                        n += 1
                    si["on_wait"] = [waits[-1]]
                out.append(inst)
            b["instructions"] = out
    return json.dumps(j).encode()


_orig_compile_bir_kernel = bass_utils.compile_bir_kernel


def _compile_bir_kernel_splitting(bir_json, tmpdir, neff_name="file.neff"):
    return _orig_compile_bir_kernel(_split_waits_in_bir(bir_json), tmpdir, neff_name)


ENABLE_LDW_OPT = False
_orig_run_command = bass_utils.run_command


def _run_command_ldw(argv, **kwargs):
    if ENABLE_LDW_OPT:
        argv = [
            a.replace("--enable-ldw-opt=false", "--enable-ldw-opt=true") for a in argv
        ]
    return _orig_run_command(argv, **kwargs)


def _install_patches():
    if not getattr(tile.TileContext, "_drain_split_installed", False):
        tile.TileContext._drain_and_barrier = _split_drain_and_barrier
        tile.TileContext._drain_split_installed = True
    if bass_utils.compile_bir_kernel is not _compile_bir_kernel_splitting:
        bass_utils.compile_bir_kernel = _compile_bir_kernel_splitting
        bass2jax.compile_bir_kernel = _compile_bir_kernel_splitting
        bass_utils.run_command = _run_command_ldw


def build_nc() -> bass.Bass:
    _install_patches()
    nc = bass.Bass(
        trn_type="TRN2", target_bir_lowering=False, debug=False, num_devices=N_CORES
    )
    # kq[h, 0:64, 0:1024] = Q^T head h ; [0:64, 1024:] = K^T head h
    # kq[h, 64:128, ...]   = the SAME data duplicated (row-group packing)
    kq = nc.dram_tensor(
        "kq", [HEADS_PER_CORE, 128, 2 * S], F16, kind="ExternalInput"
    ).ap()
    # vext[h, p, t, j]: V[h, 128*t + p, j] for j < 64, 1.0 at j == 64 (fp16)
    vext = nc.dram_tensor(
        "vext", [HEADS_PER_CORE, 128, KT, 65], F16, kind="ExternalInput"
    ).ap()
    # outu[h, 0:64, q] = unnormalized out^T ; outu[h, 64, q] = softmax sums
    outu = nc.dram_tensor(
        "outu", [HEADS_PER_CORE, 65, S], F16, kind="ExternalOutput"
    ).ap()

    with tile.TileContext(nc) as tc, ExitStack() as ctx:
        sb = ctx.enter_context(tc.tile_pool(name="sb", bufs=2))
        psS = ctx.enter_context(tc.tile_pool(name="psS", bufs=2, space="PSUM"))
        psO = ctx.enter_context(tc.tile_pool(name="psO", bufs=2, space="PSUM"))

        # pay the one-time ACT table load while the first DMAs stream
        singles = ctx.enter_context(tc.tile_pool(name="singles", bufs=1))
        warm_in = singles.tile([128, 16], F32, tag="warm_in")
        nc.vector.memset(warm_in, 0.0)
        warm_out = singles.tile([128, 16], F16, tag="warm_out")
        nc.scalar.activation(out=warm_out, in_=warm_in,
                             func=mybir.ActivationFunctionType.Exp, scale=1.0)

        kq_tiles = {}
        v_tiles = {}

        def prefetch_kq0():
            kq_s = sb.tile([128, 2 * S], F16, tag="kq", name="kq_0")
            nc.sync.dma_start(kq_s[:, 0:768], kq[0][:, 0:768])
            nc.scalar.dma_start(kq_s[:, 768:1536], kq[0][:, 768:1536])
            nc.gpsimd.dma_start(kq_s[:, 1536:2048], kq[0][:, 1536:2048])
            kq_tiles[0] = kq_s

        def prefetch_kq(h):
            kq_s = sb.tile([128, 2 * S], F16, tag="kq", name=f"kq_{h}")
            nc.sync.dma_start(kq_s[:, : S + 128], kq[h][:, : S + 128])
            nc.gpsimd.dma_start(kq_s[:, S + 128 :], kq[h][:, S + 128 :])
            kq_tiles[h] = kq_s

        def prefetch_v(h):
            v_s = sb.tile([128, KT, 65], F16, tag="v", name=f"v_{h}")
            nc.gpsimd.dma_start(v_s, vext[h])
            v_tiles[h] = v_s

        prefetch_kq0()
        prefetch_v(0)

        pending_drain = [None]

        for h in range(HEADS_PER_CORE):
            kq_s = kq_tiles.pop(h)
            v_s = v_tiles.pop(h)
            e_s = sb.tile([128, KT, 2 * 512], F16, tag="e", name=f"e_{h}")
            dve_kis = DVE_KIS[h]
            o_ps = psO.tile([65, 2 * 512], F32, tag="o", name=f"o_{h}")

            def mm2(kj):
                for c in range(2):
                    nc.tensor.matmul(
                        o_ps[:, c * 512 : (c + 1) * 512],
                        v_s[:, kj, :],
                        e_s[:, kj, c * 512 : (c + 1) * 512],
                        start=(kj == 0),
                        stop=(kj == KT - 1),
                    )

            for ki in range(KT):
                st = psS.tile([128, 2 * 512], F32, tag="st", name=f"st_{h}_{ki}")
                for c in range(2):
                    b0 = 64 * c
                    nc.tensor.matmul(
                        st[:, c * 512 : (c + 1) * 512],
                        kq_s[b0 : b0 + 64, S + ki * 128 : S + (ki + 1) * 128],
                        kq_s[b0 : b0 + 64, c * 512 : (c + 1) * 512],
                        start=True,
                        stop=True,
                        tile_position=(b0, 0),
                    )
                dst = e_s[:, ki]
                if ki in dve_kis:
                    a_t = sb.tile([128, 1024], F16, tag="exp_a")
                    nc.vector.tensor_scalar(
                        out=a_t.bitcast(I16), in0=st, scalar1=EXP_A1,
                        scalar2=EXP_B1, op0=Alu.mult, op1=Alu.add,
                    )
                    b_t = sb.tile([128, 1024], F16, tag="exp_b")
                    nc.vector.tensor_scalar(
                        out=b_t.bitcast(I16), in0=a_t.bitcast(I16),
                        scalar1=-512, scalar2=None, op0=Alu.add,
                    )
                    nc.vector.tensor_add(dst, a_t, b_t)
                else:
                    nc.scalar.activation(
                        out=dst, in_=st,
                        func=mybir.ActivationFunctionType.Exp, scale=SCALE,
                    )
                if ki == 1:
                    if h + 1 < HEADS_PER_CORE:
                        prefetch_v(h + 1)
                    # emit the previous head's drain here so the DVE queue
                    # isn't blocked on it across the head boundary
                    if pending_drain[0] is not None:
                        pending_drain[0]()
                        pending_drain[0] = None
                elif ki == 2:
                    if h + 1 < HEADS_PER_CORE:
                        prefetch_kq(h + 1)
                if ki >= LAG:
                    mm2(ki - LAG)
            for kj in range(KT - LAG, KT):
                mm2(kj)

            if h == HEADS_PER_CORE - 1:
                ou = sb.tile([65, 2 * 512], F16, tag="ou", name=f"ou_{h}")
                # split the last drain across both engines to shorten the tail
                nc.scalar.copy(out=ou[:, :512], in_=o_ps[:, :512])
                nc.vector.tensor_copy(ou[:, 512:], o_ps[:, 512:])
                nc.sync.dma_start(outu[h], ou)
            else:
                def _drain(h=h, o_ps=o_ps):
                    ou = sb.tile([65, 2 * 512], F16, tag="ou", name=f"ou_{h}")
                    nc.vector.tensor_copy(ou, o_ps)
                    nc.sync.dma_start(outu[h], ou)
                pending_drain[0] = _drain

    return nc


def _shard_inputs(queries, keys, values):
    """Full [4,16,1024,64] fp32 -> per-core kq / vext (fp16)."""
    q = np.ascontiguousarray(queries, dtype=np.float32).reshape(64, S, D)
    k = np.ascontiguousarray(keys, dtype=np.float32).reshape(64, S, D)
    v = np.ascontiguousarray(values, dtype=np.float32).reshape(64, S, D)

    qT = q.transpose(0, 2, 1)  # [64, D, S]
    kT = k.transpose(0, 2, 1)

    kq = np.empty((64, 128, 2 * S), np.float16)
    kq[:, 0:64, 0:S] = qT
    kq[:, 0:64, S:] = kT
    kq[:, 64:128, 0:S] = qT
    kq[:, 64:128, S:] = kT

    vext = np.empty((64, 128, KT, 65), np.float16)
    vext[..., 64] = 1.0
    vext[..., :64] = v.reshape(64, KT, 128, D).transpose(0, 2, 1, 3)

    in_maps = []
    for c in range(N_CORES):
        in_maps.append(
            {
                "kq": np.ascontiguousarray(kq[c * 8 : (c + 1) * 8]),
                "vext": np.ascontiguousarray(vext[c * 8 : (c + 1) * 8]),
            }
        )
    return in_maps


_CACHE = {}


def _get_nc() -> bass.Bass:
    if "nc" not in _CACHE:
        _CACHE["nc"] = build_nc()
    return _CACHE["nc"]


def run(queries, keys, values, d_k, trace=False, trace_kwargs=None):
    assert int(d_k) == D
    nc = _get_nc()
    in_maps = _shard_inputs(queries, keys, values)
    res = bass_utils.run_bass_kernel_spmd(
        nc,
        in_maps,
        core_ids=list(range(N_CORES)),
        trace=trace,
        **(trace_kwargs or {}),
    )
    outu = np.stack([r["outu"] for r in res.results]).astype(np.float32)
    # [8 cores, 8 heads, 65, S] -> normalize + transpose
    outu = outu.reshape(64, 65, S)
    out = outu[:, 0:64, :] / outu[:, 64:65, :]  # [64, D, S]
    out = np.ascontiguousarray(out.transpose(0, 2, 1)).reshape(4, 16, S, D)
    return out.astype(np.float32), res


def kernel(queries, keys, values, d_k):
    out, _ = run(queries, keys, values, d_k, trace=False)
    return out


# revision 9
# speedup vs baseline: 4.6671x; 1.0454x over previous
"""Dot-product attention on 8 Trainium2 NeuronCores — v6 (3-engine exp).

Full inputs [B=4, H=16, S=1024, D=64] fp32; 64 heads sharded 8 per core,
processed sequentially.  The kernel is pointwise-bound: every score
element must cross PSUM(fp32)->SBUF once through ScalarE (1/cyc @1.2GHz)
or VectorE (1/cyc @0.96GHz for fp32 src).  v6 balances that pass across
ScalarE (true exp, ~42 of 64 ki-tiles) and VectorE (dual-phase
Schraudolph, ~22 tiles) with GpSimdE absorbing the int16 middle step,
and removes the v5 pipeline bubbles:
  - psO (output accumulator) double-buffered (2x2 PSUM banks) so head
    h+1's MM2 no longer waits for head h's drain       (was ~1us/head)
  - score slots reduced 3->2 (2x2 banks) to fund psO   (8 banks total)
  - head 0's kq DMA split across 4 queues              (was ~4us serial)
  - last head's exp tiles split evenly + split drain   (shorter tail)

Per head h, per ki (8 k-tiles of 128):
  stage[k,q] = K^T d-major @ Q^T d-major   (fp16; Q/K duplicated in both
                                            PE row halves so the two
                                            q-half matmuls run in
                                            parallel row groups)
  E = ~exp(stage/8) fp16                   ScalarE true exp  OR
                                           VectorE A=rint(a*s+b) i16 ->
                                           GpSimdE B=A-512 ->
                                           VectorE E=f16(A)+f16(B)
  O^T+sums += [V | 1]^T @ E                (fp16, lags exp by 2 ki)
  drain: VectorE copy psum -> fp16 SBUF -> DMA out (unnormalized + sums)
Host: out[q,d] = (O^T[d,q] / sums[q])^T while gathering shards.

Toolchain notes (walrus 2026-05-04 + bass_rust skew):
 - walrus accepts at most ONE sync-wait per instruction; a JSON pass over
   the BIR hoists extra waits onto NoOps (same engine, in-order).
 - lower_dve crashes with ldw-opt enabled; keep it off.
"""

import json
from contextlib import ExitStack

import numpy as np

import concourse.bass as bass
import concourse.bass2jax as bass2jax
import concourse.mybir as mybir
import concourse.tile as tile
from concourse import bass_utils
from concourse.vector_clock import ScopedClock

F32 = mybir.dt.float32
F16 = mybir.dt.float16
I16 = mybir.dt.int16
Alu = mybir.AluOpType

N_CORES = 8
HEADS_PER_CORE = 8
S = 1024
D = 64
KT = S // 128  # 8 k-tiles per head

SCALE = 0.125  # 1/sqrt(64)

# dual-phase Schraudolph constants: A = rint(a1*s + b1) int16;
# E = fp16_bits(A) + fp16_bits(A-512) approximates exp(s*SCALE)
# with max rel err ~1.1% and unit gain.
EXP_A1 = 184.6649627685547
EXP_B1 = 14517.731933593746

# which ki-stages per head use the VectorE fast-exp path
DVE_KIS = {
    0: (0, 1, 4),
    1: (2, 6),
    2: (2, 6),
    3: (2, 6),
    4: (2, 6),
    5: (2, 6),
    6: (2, 6),
    7: (1, 3, 5),
}
# MM2 trails exp by this many ki (flat across head boundaries).  Must be
# large enough that every MM2 in the tensor stream is already runnable
# when emitted — a stale-E MM2 sitting ahead of an MM1 inflates the
# engine-clock wait threshold of the exp depending on that MM1.
LAG = 4

_DRAIN_MAX_WAITS = 1


def _split_drain_and_barrier(self, tick_clock, wait_clock):
    nc = self.nc
    drain_inst = nc.sync.drain()
    wait_clock.add_sem_waits(
        drain_inst.ins, ScopedClock({None: tick_clock.global_clock})
    )
    si = drain_inst.ins.sync_info
    if si is not None and si.on_wait and len(si.on_wait) > _DRAIN_MAX_WAITS:
        waits = list(si.on_wait)
        updates = list(si.on_update or [])
        drain_inst.ins.sync_info = mybir.SyncInfo(
            on_wait=waits[:_DRAIN_MAX_WAITS], on_update=[]
        )
        rest = waits[_DRAIN_MAX_WAITS:]
        for i in range(0, len(rest), _DRAIN_MAX_WAITS):
            extra = nc.sync.drain()
            extra.ins.sync_info = mybir.SyncInfo(
                on_wait=rest[i : i + _DRAIN_MAX_WAITS],
                on_update=updates if i + _DRAIN_MAX_WAITS >= len(rest) else [],
            )
    nc.all_engine_barrier()
    assert self.sems is not None
    popped = nc._tile_sem_poison_stack.pop()
    assert popped is self._sem_poison
    nc.clear_and_free_semaphores(list(self.sems.allocated().values()))
    nc.all_engine_barrier()


def _split_waits_in_bir(bir_json: bytes) -> bytes:
    """Hoist extra sync-waits onto NoOps inserted immediately before the
    owning instruction (same engine, in-order => semantics unchanged)."""
    j = json.loads(bir_json)
    n = 0
    for f in j["functions"]:
        for b in f["blocks"]:
            out = []
            for inst in b["instructions"]:
                si = inst.get("sync_info")
                waits = (si or {}).get("on_wait") or []
                if len(waits) > 1:
                    for w in waits[:-1]:
                        out.append(
                            {
                                "debug": inst.get("debug", 0),
                                "engine": inst["engine"],
                                "ins": [],
                                "outs": [],
                                "name": f"{inst['name']}-wsplit{n}",
                                "opcode": "NoOp",
                                "sync_info": {"on_update": [], "on_wait": [w]},
                            }
                        )
                        n += 1
                    si["on_wait"] = [waits[-1]]
                out.append(inst)
            b["instructions"] = out
    return json.dumps(j).encode()


_orig_compile_bir_kernel = bass_utils.compile_bir_kernel


def _compile_bir_kernel_splitting(bir_json, tmpdir, neff_name="file.neff"):
    return _orig_compile_bir_kernel(_split_waits_in_bir(bir_json), tmpdir, neff_name)


ENABLE_LDW_OPT = False
_orig_run_command = bass_utils.run_command


def _run_command_ldw(argv, **kwargs):
    if ENABLE_LDW_OPT:
        argv = [
            a.replace("--enable-ldw-opt=false", "--enable-ldw-opt=true") for a in argv
        ]
    return _orig_run_command(argv, **kwargs)


def _install_patches():
    if not getattr(tile.TileContext, "_drain_split_installed", False):
        tile.TileContext._drain_and_barrier = _split_drain_and_barrier
        tile.TileContext._drain_split_installed = True
    if bass_utils.compile_bir_kernel is not _compile_bir_kernel_splitting:
        bass_utils.compile_bir_kernel = _compile_bir_kernel_splitting
        bass2jax.compile_bir_kernel = _compile_bir_kernel_splitting
        bass_utils.run_command = _run_command_ldw


def build_nc() -> bass.Bass:
    _install_patches()
    nc = bass.Bass(
        trn_type="TRN2", target_bir_lowering=False, debug=False, num_devices=N_CORES
    )
    # kq[h, 0:64, 0:1024] = Q^T head h ; [0:64, 1024:] = K^T head h
    # kq[h, 64:128, ...]   = the SAME data duplicated (row-group packing)
    kq = nc.dram_tensor(
        "kq", [HEADS_PER_CORE, 128, 2 * S], F16, kind="ExternalInput"
    ).ap()
    # vext[h, p, t, j]: V[h, 128*t + p, j] for j < 64, 1.0 at j == 64 (fp16)
    vext = nc.dram_tensor(
        "vext", [HEADS_PER_CORE, 128, KT, 65], F16, kind="ExternalInput"
    ).ap()
    # outu[h, 0:64, q] = unnormalized out^T ; outu[h, 64, q] = softmax sums
    outu = nc.dram_tensor(
        "outu", [HEADS_PER_CORE, 65, S], F16, kind="ExternalOutput"
    ).ap()

    with tile.TileContext(nc) as tc, ExitStack() as ctx:
        sb = ctx.enter_context(tc.tile_pool(name="sb", bufs=2))
        psS = ctx.enter_context(tc.tile_pool(name="psS", bufs=2, space="PSUM"))
        psO = ctx.enter_context(tc.tile_pool(name="psO", bufs=2, space="PSUM"))

        # pay the one-time ACT table load while the first DMAs stream
        singles = ctx.enter_context(tc.tile_pool(name="singles", bufs=1))
        warm_in = singles.tile([128, 16], F32, tag="warm_in")
        nc.vector.memset(warm_in, 0.0)
        warm_out = singles.tile([128, 16], F16, tag="warm_out")
        nc.scalar.activation(out=warm_out, in_=warm_in,
                             func=mybir.ActivationFunctionType.Exp, scale=1.0)

        kq_tiles = {}
        v_tiles = {}

        def prefetch_kq0():
            kq_s = sb.tile([128, 2 * S], F16, tag="kq", name="kq_0")
            nc.sync.dma_start(kq_s[:, 0:768], kq[0][:, 0:768])
            nc.scalar.dma_start(kq_s[:, 768:1536], kq[0][:, 768:1536])
            nc.gpsimd.dma_start(kq_s[:, 1536:2048], kq[0][:, 1536:2048])
            kq_tiles[0] = kq_s

        def prefetch_kq(h):
            kq_s = sb.tile([128, 2 * S], F16, tag="kq", name=f"kq_{h}")
            nc.sync.dma_start(kq_s[:, : S + 128], kq[h][:, : S + 128])
            nc.gpsimd.dma_start(kq_s[:, S + 128 :], kq[h][:, S + 128 :])
            kq_tiles[h] = kq_s

        def prefetch_v(h):
            v_s = sb.tile([128, KT, 65], F16, tag="v", name=f"v_{h}")
            nc.gpsimd.dma_start(v_s, vext[h])
            v_tiles[h] = v_s

        prefetch_kq0()
        prefetch_v(0)

        heads = {}

        def mm2(hj, kj):
            hs = heads[hj]
            if kj == 0:
                hs["o_ps"] = psO.tile([65, 2 * 512], F32, tag="o", name=f"o_{hj}")
            for c in range(2):
                nc.tensor.matmul(
                    hs["o_ps"][:, c * 512 : (c + 1) * 512],
                    hs["v_s"][:, kj, :],
                    hs["e_s"][:, kj, c * 512 : (c + 1) * 512],
                    start=(kj == 0),
                    stop=(kj == KT - 1),
                )

        def drain(hj, split):
            o_ps = heads[hj]["o_ps"]
            ou = sb.tile([65, 2 * 512], F16, tag="ou", name=f"ou_{hj}")
            if split:
                # split across both engines to shorten the kernel tail
                nc.scalar.copy(out=ou[:, :512], in_=o_ps[:, :512])
                nc.vector.tensor_copy(ou[:, 512:], o_ps[:, 512:])
            else:
                nc.vector.tensor_copy(ou, o_ps)
            nc.sync.dma_start(outu[hj], ou)

        NG = HEADS_PER_CORE * KT
        for g in range(NG + LAG):
            if g < NG:
                h, ki = divmod(g, KT)
                if ki == 0:
                    heads[h] = {
                        "kq_s": kq_tiles.pop(h),
                        "v_s": v_tiles.pop(h),
                        "e_s": sb.tile([128, KT, 2 * 512], F16, tag="e",
                                       name=f"e_{h}"),
                    }
                kq_s = heads[h]["kq_s"]
                st = psS.tile([128, 2 * 512], F32, tag="st", name=f"st_{h}_{ki}")
                for c in range(2):
                    b0 = 64 * c
                    nc.tensor.matmul(
                        st[:, c * 512 : (c + 1) * 512],
                        kq_s[b0 : b0 + 64, S + ki * 128 : S + (ki + 1) * 128],
                        kq_s[b0 : b0 + 64, c * 512 : (c + 1) * 512],
                        start=True,
                        stop=True,
                        tile_position=(b0, 0),
                    )
                dst = heads[h]["e_s"][:, ki]
                if ki in DVE_KIS[h]:
                    a_t = sb.tile([128, 1024], F16, tag="exp_a")
                    nc.vector.tensor_scalar(
                        out=a_t.bitcast(I16), in0=st, scalar1=EXP_A1,
                        scalar2=EXP_B1, op0=Alu.mult, op1=Alu.add,
                    )
                    b_t = sb.tile([128, 1024], F16, tag="exp_b")
                    nc.vector.tensor_scalar(
                        out=b_t.bitcast(I16), in0=a_t.bitcast(I16),
                        scalar1=-512, scalar2=None, op0=Alu.add,
                    )
                    nc.vector.tensor_add(dst, a_t, b_t)
                else:
                    nc.scalar.activation(
                        out=dst, in_=st,
                        func=mybir.ActivationFunctionType.Exp, scale=SCALE,
                    )
                if ki == 1 and h + 1 < HEADS_PER_CORE:
                    prefetch_v(h + 1)
                elif ki == 2 and h + 1 < HEADS_PER_CORE:
                    prefetch_kq(h + 1)
                elif ki == 5 and h > 0:
                    drain(h - 1, split=False)
            if g >= LAG:
                mm2(*divmod(g - LAG, KT))
        drain(HEADS_PER_CORE - 1, split=True)

    return nc


def _shard_inputs(queries, keys, values):
    """Full [4,16,1024,64] fp32 -> per-core kq / vext (fp16)."""
    q = np.ascontiguousarray(queries, dtype=np.float32).reshape(64, S, D)
    k = np.ascontiguousarray(keys, dtype=np.float32).reshape(64, S, D)
    v = np.ascontiguousarray(values, dtype=np.float32).reshape(64, S, D)

    qT = q.transpose(0, 2, 1)  # [64, D, S]
    kT = k.transpose(0, 2, 1)

    kq = np.empty((64, 128, 2 * S), np.float16)
    kq[:, 0:64, 0:S] = qT
    kq[:, 0:64, S:] = kT
    kq[:, 64:128, 0:S] = qT
    kq[:, 64:128, S:] = kT

    vext = np.empty((64, 128, KT, 65), np.float16)
    vext[..., 64] = 1.0
    vext[..., :64] = v.reshape(64, KT, 128, D).transpose(0, 2, 1, 3)

    in_maps = []
    for c in range(N_CORES):
        in_maps.append(
            {
                "kq": np.ascontiguousarray(kq[c * 8 : (c + 1) * 8]),
                "vext": np.ascontiguousarray(vext[c * 8 : (c + 1) * 8]),
            }
        )
    return in_maps


_CACHE = {}


def _get_nc() -> bass.Bass:
    if "nc" not in _CACHE:
        _CACHE["nc"] = build_nc()
    return _CACHE["nc"]


def run(queries, keys, values, d_k, trace=False, trace_kwargs=None):
    assert int(d_k) == D
    nc = _get_nc()
    in_maps = _shard_inputs(queries, keys, values)
    res = bass_utils.run_bass_kernel_spmd(
        nc,
        in_maps,
        core_ids=list(range(N_CORES)),
        trace=trace,
        **(trace_kwargs or {}),
    )
    outu = np.stack([r["outu"] for r in res.results]).astype(np.float32)
    # [8 cores, 8 heads, 65, S] -> normalize + transpose
    outu = outu.reshape(64, 65, S)
    out = outu[:, 0:64, :] / outu[:, 64:65, :]  # [64, D, S]
    out = np.ascontiguousarray(out.transpose(0, 2, 1)).reshape(4, 16, S, D)
    return out.astype(np.float32), res


def kernel(queries, keys, values, d_k):
    out, _ = run(queries, keys, values, d_k, trace=False)
    return out


# revision 11
# speedup vs baseline: 4.8463x; 1.0384x over previous
"""Dot-product attention on 8 Trainium2 NeuronCores — v6 (3-engine exp).

Full inputs [B=4, H=16, S=1024, D=64] fp32; 64 heads sharded 8 per core,
processed sequentially.  The kernel is pointwise-bound: every score
element must cross PSUM(fp32)->SBUF once through ScalarE (1/cyc @1.2GHz)
or VectorE (1/cyc @0.96GHz for fp32 src).  v6 balances that pass across
ScalarE (true exp, ~42 of 64 ki-tiles) and VectorE (dual-phase
Schraudolph, ~22 tiles) with GpSimdE absorbing the int16 middle step,
and removes the v5 pipeline bubbles:
  - psO (output accumulator) double-buffered (2x2 PSUM banks) so head
    h+1's MM2 no longer waits for head h's drain       (was ~1us/head)
  - score slots reduced 3->2 (2x2 banks) to fund psO   (8 banks total)
  - head 0's kq DMA split across 4 queues              (was ~4us serial)
  - last head's exp tiles split evenly + split drain   (shorter tail)

Per head h, per ki (8 k-tiles of 128):
  stage[k,q] = K^T d-major @ Q^T d-major   (fp16; Q/K duplicated in both
                                            PE row halves so the two
                                            q-half matmuls run in
                                            parallel row groups)
  E = ~exp(stage/8) fp16                   ScalarE true exp  OR
                                           VectorE A=rint(a*s+b) i16 ->
                                           GpSimdE B=A-512 ->
                                           VectorE E=f16(A)+f16(B)
  O^T+sums += [V | 1]^T @ E                (fp16, lags exp by 2 ki)
  drain: VectorE copy psum -> fp16 SBUF -> DMA out (unnormalized + sums)
Host: out[q,d] = (O^T[d,q] / sums[q])^T while gathering shards.

Toolchain notes (walrus 2026-05-04 + bass_rust skew):
 - walrus accepts at most ONE sync-wait per instruction; a JSON pass over
   the BIR hoists extra waits onto NoOps (same engine, in-order).
 - lower_dve crashes with ldw-opt enabled; keep it off.
"""

import json
from contextlib import ExitStack

import numpy as np

import concourse.bass as bass
import concourse.bass2jax as bass2jax
import concourse.mybir as mybir
import concourse.tile as tile
from concourse import bass_utils
from concourse.vector_clock import ScopedClock

F32 = mybir.dt.float32
F16 = mybir.dt.float16
I16 = mybir.dt.int16
Alu = mybir.AluOpType

N_CORES = 8
HEADS_PER_CORE = 8
S = 1024
D = 64
KT = S // 128  # 8 k-tiles per head

SCALE = 0.125  # 1/sqrt(64)

# dual-phase Schraudolph constants: A = rint(a1*s + b1) int16;
# E = fp16_bits(A) + fp16_bits(A-512) approximates exp(s*SCALE)
# with max rel err ~1.1% and unit gain.
EXP_A1 = 184.6649627685547
EXP_B1 = 14517.731933593746

# which ki-stages per head use the VectorE fast-exp path
DVE_KIS = {
    0: (0, 1, 4),
    1: (2, 6),
    2: (2, 6),
    3: (2, 6),
    4: (2, 6),
    5: (2, 6),
    6: (2, 6),
    7: (1, 3, 5),
}
# MM2 trails exp by this many ki (flat across head boundaries).  Must be
# large enough that every MM2 in the tensor stream is already runnable
# when emitted — a stale-E MM2 sitting ahead of an MM1 inflates the
# engine-clock wait threshold of the exp depending on that MM1.
LAG = 4

_DRAIN_MAX_WAITS = 1


def _split_drain_and_barrier(self, tick_clock, wait_clock):
    nc = self.nc
    drain_inst = nc.sync.drain()
    wait_clock.add_sem_waits(
        drain_inst.ins, ScopedClock({None: tick_clock.global_clock})
    )
    si = drain_inst.ins.sync_info
    if si is not None and si.on_wait and len(si.on_wait) > _DRAIN_MAX_WAITS:
        waits = list(si.on_wait)
        updates = list(si.on_update or [])
        drain_inst.ins.sync_info = mybir.SyncInfo(
            on_wait=waits[:_DRAIN_MAX_WAITS], on_update=[]
        )
        rest = waits[_DRAIN_MAX_WAITS:]
        for i in range(0, len(rest), _DRAIN_MAX_WAITS):
            extra = nc.sync.drain()
            extra.ins.sync_info = mybir.SyncInfo(
                on_wait=rest[i : i + _DRAIN_MAX_WAITS],
                on_update=updates if i + _DRAIN_MAX_WAITS >= len(rest) else [],
            )
    nc.all_engine_barrier()
    assert self.sems is not None
    popped = nc._tile_sem_poison_stack.pop()
    assert popped is self._sem_poison
    nc.clear_and_free_semaphores(list(self.sems.allocated().values()))
    nc.all_engine_barrier()


def _split_waits_in_bir(bir_json: bytes) -> bytes:
    """Hoist extra sync-waits onto NoOps inserted immediately before the
    owning instruction (same engine, in-order => semantics unchanged)."""
    j = json.loads(bir_json)
    n = 0
    for f in j["functions"]:
        for b in f["blocks"]:
            out = []
            for inst in b["instructions"]:
                si = inst.get("sync_info")
                waits = (si or {}).get("on_wait") or []
                if len(waits) > 1:
                    for w in waits[:-1]:
                        out.append(
                            {
                                "debug": inst.get("debug", 0),
                                "engine": inst["engine"],
                                "ins": [],
                                "outs": [],
                                "name": f"{inst['name']}-wsplit{n}",
                                "opcode": "NoOp",
                                "sync_info": {"on_update": [], "on_wait": [w]},
                            }
                        )
                        n += 1
                    si["on_wait"] = [waits[-1]]
                out.append(inst)
            b["instructions"] = out
    return json.dumps(j).encode()


_orig_compile_bir_kernel = bass_utils.compile_bir_kernel


def _compile_bir_kernel_splitting(bir_json, tmpdir, neff_name="file.neff"):
    return _orig_compile_bir_kernel(_split_waits_in_bir(bir_json), tmpdir, neff_name)


ENABLE_LDW_OPT = False
_orig_run_command = bass_utils.run_command


def _run_command_ldw(argv, **kwargs):
    if ENABLE_LDW_OPT:
        argv = [
            a.replace("--enable-ldw-opt=false", "--enable-ldw-opt=true") for a in argv
        ]
    return _orig_run_command(argv, **kwargs)


def _install_patches():
    if not getattr(tile.TileContext, "_drain_split_installed", False):
        tile.TileContext._drain_and_barrier = _split_drain_and_barrier
        tile.TileContext._drain_split_installed = True
    if bass_utils.compile_bir_kernel is not _compile_bir_kernel_splitting:
        bass_utils.compile_bir_kernel = _compile_bir_kernel_splitting
        bass2jax.compile_bir_kernel = _compile_bir_kernel_splitting
        bass_utils.run_command = _run_command_ldw


def build_nc() -> bass.Bass:
    _install_patches()
    nc = bass.Bass(
        trn_type="TRN2", target_bir_lowering=False, debug=False, num_devices=N_CORES
    )
    # kq[h, 0:64, 0:1024] = Q^T head h ; [0:64, 1024:] = K^T head h
    # kq[h, 64:128, ...]   = the SAME data duplicated (row-group packing)
    kq = nc.dram_tensor(
        "kq", [HEADS_PER_CORE, 128, 2 * S], F16, kind="ExternalInput"
    ).ap()
    # vext[h, p, t, j]: V[h, 128*t + p, j] for j < 64, 1.0 at j == 64 (fp16)
    vext = nc.dram_tensor(
        "vext", [HEADS_PER_CORE, 128, KT, 65], F16, kind="ExternalInput"
    ).ap()
    # outu[h, 0:64, q] = unnormalized out^T ; outu[h, 64, q] = softmax sums
    outu = nc.dram_tensor(
        "outu", [HEADS_PER_CORE, 65, S], F16, kind="ExternalOutput"
    ).ap()

    with tile.TileContext(nc) as tc, ExitStack() as ctx:
        sb = ctx.enter_context(tc.tile_pool(name="sb", bufs=2))
        psS = ctx.enter_context(tc.tile_pool(name="psS", bufs=3, space="PSUM"))
        psO = ctx.enter_context(tc.tile_pool(name="psO", bufs=1, space="PSUM"))

        # pay the one-time ACT table load while the first DMAs stream
        singles = ctx.enter_context(tc.tile_pool(name="singles", bufs=1))
        warm_in = singles.tile([128, 16], F32, tag="warm_in")
        nc.vector.memset(warm_in, 0.0)
        warm_out = singles.tile([128, 16], F16, tag="warm_out")
        nc.scalar.activation(out=warm_out, in_=warm_in,
                             func=mybir.ActivationFunctionType.Exp, scale=1.0)

        kq_tiles = {}
        v_tiles = {}

        def prefetch_kq0():
            kq_s = sb.tile([128, 2 * S], F16, tag="kq", name="kq_0")
            nc.sync.dma_start(kq_s[:, 0:768], kq[0][:, 0:768])
            nc.scalar.dma_start(kq_s[:, 768:1536], kq[0][:, 768:1536])
            nc.gpsimd.dma_start(kq_s[:, 1536:2048], kq[0][:, 1536:2048])
            kq_tiles[0] = kq_s

        def prefetch_kq(h):
            kq_s = sb.tile([128, 2 * S], F16, tag="kq", name=f"kq_{h}")
            nc.sync.dma_start(kq_s[:, : S + 128], kq[h][:, : S + 128])
            nc.gpsimd.dma_start(kq_s[:, S + 128 :], kq[h][:, S + 128 :])
            kq_tiles[h] = kq_s

        def prefetch_v(h):
            v_s = sb.tile([128, KT, 65], F16, tag="v", name=f"v_{h}")
            nc.gpsimd.dma_start(v_s, vext[h])
            v_tiles[h] = v_s

        prefetch_kq0()
        prefetch_v(0)

        heads = {}

        def mm2(hj, kj):
            hs = heads[hj]
            if kj == 0:
                hs["o_ps"] = psO.tile([65, 2 * 512], F32, tag="o", name=f"o_{hj}")
            for c in range(2):
                nc.tensor.matmul(
                    hs["o_ps"][:, c * 512 : (c + 1) * 512],
                    hs["v_s"][:, kj, :],
                    hs["e_s"][:, kj, c * 512 : (c + 1) * 512],
                    start=(kj == 0),
                    stop=(kj == KT - 1),
                )

        def drain(hj, split):
            o_ps = heads[hj]["o_ps"]
            ou = sb.tile([65, 2 * 512], F16, tag="ou", name=f"ou_{hj}")
            if split:
                # split across both engines to shorten the kernel tail
                nc.scalar.copy(out=ou[:, :512], in_=o_ps[:, :512])
                nc.vector.tensor_copy(ou[:, 512:], o_ps[:, 512:])
            else:
                nc.vector.tensor_copy(ou, o_ps)
            nc.sync.dma_start(outu[hj], ou)

        NG = HEADS_PER_CORE * KT
        for g in range(NG + LAG):
            if g < NG:
                h, ki = divmod(g, KT)
                if ki == 0:
                    heads[h] = {
                        "kq_s": kq_tiles.pop(h),
                        "v_s": v_tiles.pop(h),
                        "e_s": sb.tile([128, KT, 2 * 512], F16, tag="e",
                                       name=f"e_{h}"),
                    }
                kq_s = heads[h]["kq_s"]
                st = psS.tile([128, 2 * 512], F32, tag="st", name=f"st_{h}_{ki}")
                for c in range(2):
                    b0 = 64 * c
                    nc.tensor.matmul(
                        st[:, c * 512 : (c + 1) * 512],
                        kq_s[b0 : b0 + 64, S + ki * 128 : S + (ki + 1) * 128],
                        kq_s[b0 : b0 + 64, c * 512 : (c + 1) * 512],
                        start=True,
                        stop=True,
                        tile_position=(b0, 0),
                    )
                dst = heads[h]["e_s"][:, ki]
                if ki in DVE_KIS[h]:
                    a_t = sb.tile([128, 1024], F16, tag="exp_a")
                    nc.vector.tensor_scalar(
                        out=a_t.bitcast(I16), in0=st, scalar1=EXP_A1,
                        scalar2=EXP_B1, op0=Alu.mult, op1=Alu.add,
                    )
                    b_t = sb.tile([128, 1024], F16, tag="exp_b")
                    nc.vector.tensor_scalar(
                        out=b_t.bitcast(I16), in0=a_t.bitcast(I16),
                        scalar1=-512, scalar2=None, op0=Alu.add,
                    )
                    nc.vector.tensor_add(dst, a_t, b_t)
                else:
                    nc.scalar.activation(
                        out=dst, in_=st,
                        func=mybir.ActivationFunctionType.Exp, scale=SCALE,
                    )
                if ki == 1 and h + 1 < HEADS_PER_CORE:
                    prefetch_v(h + 1)
                elif ki == 2 and h + 1 < HEADS_PER_CORE:
                    prefetch_kq(h + 1)
                elif ki == 4 and h > 0:
                    # lands in the DVE idle window between its ki2 and ki6
                    # chains; MM2(h-1,7) retired one ki earlier.
                    drain(h - 1, split=False)
            if g >= LAG:
                hj, kj = divmod(g - LAG, KT)
                if kj == 1:
                    # kj=0 deferred one ki so the tensor stream never waits
                    # on the previous head's drain (psO is single-buffered)
                    mm2(hj, 0)
                    mm2(hj, 1)
                elif kj != 0:
                    mm2(hj, kj)
        drain(HEADS_PER_CORE - 1, split=True)

    return nc


def _shard_inputs(queries, keys, values):
    """Full [4,16,1024,64] fp32 -> per-core kq / vext (fp16)."""
    q = np.ascontiguousarray(queries, dtype=np.float32).reshape(64, S, D)
    k = np.ascontiguousarray(keys, dtype=np.float32).reshape(64, S, D)
    v = np.ascontiguousarray(values, dtype=np.float32).reshape(64, S, D)

    qT = q.transpose(0, 2, 1)  # [64, D, S]
    kT = k.transpose(0, 2, 1)

    kq = np.empty((64, 128, 2 * S), np.float16)
    kq[:, 0:64, 0:S] = qT
    kq[:, 0:64, S:] = kT
    kq[:, 64:128, 0:S] = qT
    kq[:, 64:128, S:] = kT

    vext = np.empty((64, 128, KT, 65), np.float16)
    vext[..., 64] = 1.0
    vext[..., :64] = v.reshape(64, KT, 128, D).transpose(0, 2, 1, 3)

    in_maps = []
    for c in range(N_CORES):
        in_maps.append(
            {
                "kq": np.ascontiguousarray(kq[c * 8 : (c + 1) * 8]),
                "vext": np.ascontiguousarray(vext[c * 8 : (c + 1) * 8]),
            }
        )
    return in_maps


_CACHE = {}


def _get_nc() -> bass.Bass:
    if "nc" not in _CACHE:
        _CACHE["nc"] = build_nc()
    return _CACHE["nc"]


def run(queries, keys, values, d_k, trace=False, trace_kwargs=None):
    assert int(d_k) == D
    nc = _get_nc()
    in_maps = _shard_inputs(queries, keys, values)
    res = bass_utils.run_bass_kernel_spmd(
        nc,
        in_maps,
        core_ids=list(range(N_CORES)),
        trace=trace,
        **(trace_kwargs or {}),
    )
    outu = np.stack([r["outu"] for r in res.results]).astype(np.float32)
    # [8 cores, 8 heads, 65, S] -> normalize + transpose
    outu = outu.reshape(64, 65, S)
    out = outu[:, 0:64, :] / outu[:, 64:65, :]  # [64, D, S]
    out = np.ascontiguousarray(out.transpose(0, 2, 1)).reshape(4, 16, S, D)
    return out.astype(np.float32), res


def kernel(queries, keys, values, d_k):
    out, _ = run(queries, keys, values, d_k, trace=False)
    return out
